# revision 20
# baseline (speedup 1.0000x reference)
"""CRF Viterbi decode kernel for Trainium2 (8 NeuronCores, data-parallel).

Problem: inputs [4096, 512, 48] f32, transitions [48, 48] f32, seq_lens [4096] i32.
Output: one-hot of the Viterbi path, [4096, 512, 48] f32 (bit-exact vs the
fp32 jax reference, including first-index argmax tie-breaks).

Design (v2 — fused backpointer scan)
------------------------------------
- Data parallel over batch: 8 cores x 4 blocks of 128 examples (partitions).
  Examples globally sorted by seq_len (desc); block position g on every core
  holds similar lengths and runs a static step count L[g].
- Forward DP on DVE via ONE custom op per step (VITERBI_SCAN_BP): pages of
  49 streaming the candidates in REVERSED tag coords (kappa = 47 - i,
  page a = 47 - j). Per page the op emits the running max stream, the page
  max M at slot 47, and the FIRST-INDEX argmax (1-based kappa coord) at
  slot 48 — exact first-i tie-break falls out of record-detection on the
  reversed stream. IEEE fp32 adds -> bit-exact scores vs the reference.
- s_t = M + x_t on Pool (GPSIMD); per-(example,step) freeze handled by exact
  multiplicative 0/1 masking on Pool. Backpointer rows stream to DRAM.
- Backtrack: ONE tiny DVE op per step (VITERBI_GATHER): accumulate
  select(iota==dotval, bp_row, 0) -> gathered backpointer, which in the
  reversed coordinate system IS the next dotval directly. One-hot output
  columns built on Pool (is_eq against a reversed iota), so the Vector
  engine runs scans + 48-element gathers only.
- Emission: forward blocks co-advance in pairs so each block's Pool add runs
  in the shadow of the other's scan; backtracks pump between scans.
"""

import sys

sys.path.insert(0, "/opt/trn_rl_repo")

import numpy as np

N = 48
NP1 = N + 1  # padded page width
TT = 512
BB = 4096
NCORES = 8
PB = 128  # examples per block (partitions)
NBLK = 4  # blocks per core
CHS = 32  # x / backpointer chunk (steps)
CHO = 32  # output chunk (steps)
NEG = -3.4e38


# --------------------------------------------------------------------------
# custom DVE ops
# --------------------------------------------------------------------------
def _patch_framework():
    import concourse.dve_spec as ds
    from concourse.dve_spec import AluOp

    # segmented-scan page reset: at each sub-dim boundary the accumulator
    # restarts from the boundary element's expr.
    if not getattr(ds, "_ant_seg_reset_patched", False):
        _orig = ds._scan_overrides

        def _patched(scans, node_stage):
            seed, step = _orig(scans, node_stage)
            for sc in scans:
                if getattr(sc, "_ant_seg_reset", False):
                    d = node_stage[sc]
                    step[d] = ds._Stage(AluOp.BYPASS, sc.expr)
            return seed, step

        ds._scan_overrides = _patched
        ds._ant_seg_reset_patched = True

    # hand placement hook (the stock list scheduler mis-orders the
    # counter scan and inserts a cond shim, overflowing the 8 stages).
    if not getattr(ds, "_ant_sched_patched", False):
        _orig_sched = ds._schedule

        def _sched(body, n_stages):
            hs = getattr(ds, "_ant_hand_schedules", {}).get(id(body))
            if hs is not None:
                return dict(hs[0]), list(hs[1]), {}
            return _orig_sched(body, n_stages)

        ds._schedule = _sched
        ds._ant_sched_patched = True
        ds._ant_hand_schedules = {}


def _raw_scan(op, expr, init=None):
    """Scan node bypassing __post_init__ (its nested-scan validator rejects
    cross-scan dataflow that the datapath supports fine)."""
    from concourse.dve_spec import Scan

    sc = Scan.__new__(Scan)
    object.__setattr__(sc, "op", op)
    object.__setattr__(sc, "expr", expr)
    object.__setattr__(sc, "init", init)
    object.__setattr__(sc, "_subdim_step", None)
    object.__setattr__(sc, "_ant_seg_reset", True)
    return sc


def _register_op(name, spec, subdim):
    import concourse.dve_ops as dops
    from concourse.dve_uop import DveOpSpec
    import concourse.dve_spec as ds

    shas = {}
    for ver in ("v3", "v4"):
        uops = ds.lower(spec, ver=ver)
        shas[ver] = DveOpSpec(
            name=name, opcode=1, uops=uops, rd1_en=dops.has_src1(spec)
        ).sha(ver)
    op = dops.DveOp(name, spec, subdim=subdim, uops_sha=shas)
    dops.OPS.append(op)
    dops.CUSTOM_DVE_SPECS[op.name] = op.spec
    dops._SUB_OPCODE_FOR_NAME[op.name] = dops._CUSTOM_DVE_ROW_BASE + len(dops.OPS) - 1
    return op


def _make_scan_bp_op():
    """Segmented (pages of NP1) op on reversed-coord candidate streams:
      v    = Src0 + Src1                   (T_pad + s broadcast)
      m    = seg-scan(MAX, v)              running page max
      idxc = seg-scan(ADD, 1, init=0)      1..49 within page
      flag = v >= m                        record-or-tie (v == m)
      isel = select(flag, idxc, -3.4e38)
      amax = seg-scan(MAX, isel)           last flagged idxc = first-argmax
      out  = select(idxc >= s0, amax, m)   s0 = 49.0: slot 48 carries amax
    Page layout of out: [m stream ... M at slot 47, bp (1-based kappa) at 48].
    """
    import concourse.dve_ops as dops
    import concourse.dve_spec as ds
    from concourse.dve_spec import Spec, Src0, Src1, C0, One, MaxNeg, AluOp, Bin, Tri

    for op in dops.OPS:
        if op.name == "VITERBI_SCAN_BP":
            return op
    _patch_framework()

    v = Bin(AluOp.ADD, Src0, Src1)
    m = _raw_scan(AluOp.MAX, v)
    idxc = _raw_scan(AluOp.ADD, One, init=Bin(AluOp.SUBTRACT, One, One))
    flag = Bin(AluOp.IS_GE, v, m)
    isel = Tri(AluOp.SELECT, flag, idxc, MaxNeg)
    amax = _raw_scan(AluOp.MAX, isel)
    pos = Bin(AluOp.IS_GE, idxc, C0)
    outsel = Tri(AluOp.SELECT, pos, amax, m)

    ds._ant_hand_schedules[id(outsel)] = (
        {v: 0, m: 1, idxc: 2, flag: 3, isel: 4, amax: 5, pos: 6, outsel: 7},
        [Src0, Src1, One, MaxNeg, C0],
    )

    def _ref(in0, in1, s0, s1, imm2):
        v = (np.asarray(in0, np.float32) + np.asarray(in1, np.float32)).astype(
            np.float32
        )
        m = np.maximum.accumulate(v, axis=-1)
        n = v.shape[-1]
        idxc = np.arange(1, n + 1, dtype=np.float32)
        srt = [1] * (v.ndim - 1) + [n]
        isel = np.where(v >= m, idxc.reshape(srt), np.float32(NEG)).astype(np.float32)
        amax = np.maximum.accumulate(isel, axis=-1)
        return np.where(idxc.reshape(srt) >= np.float32(s0), amax, m).astype(
            np.float32
        )

    return _register_op("VITERBI_SCAN_BP", Spec(body=outsel, reference=_ref), True)


def _make_pick_op(name, accum_op):
    """out-stream = select(Src0 == s0, Src1, 0); accum_out = accum over it.
    With accum=ADD and Src0 an iota matched exactly once: a per-partition
    gather of Src1 at index s0. With accum=MAX: first-index argmax helper.
    """
    import concourse.dve_ops as dops
    from concourse.dve_spec import Spec, Src0, Src1, C0, Zero, AluOp, Bin, Tri

    for op in dops.OPS:
        if op.name == name:
            return op
    _patch_framework()

    sel = Tri(AluOp.SELECT, Bin(AluOp.IS_EQ, Src0, C0), Src1, Zero)

    def _ref(in0, in1, s0, s1, imm2):
        in0 = np.asarray(in0, np.float32)
        in1 = np.asarray(in1, np.float32)
        s0a = np.asarray(s0, np.float32)
        while s0a.ndim < in0.ndim:
            s0a = s0a[..., None]
        out = np.where(in0 == s0a, in1, np.float32(0.0)).astype(np.float32)
        if accum_op == AluOp.ADD:
            acc = out.sum(axis=-1, dtype=np.float32)
        else:
            acc = out.max(axis=-1)
        return out, acc.astype(np.float32)

    return _register_op(name, Spec(body=sel, accum=accum_op, reference=_ref), False)


# --------------------------------------------------------------------------
# device program
# --------------------------------------------------------------------------
def _build_nc(L, minL, t_total):
    import concourse.tile as tile
    import concourse.bacc as bacc
    from concourse import mybir

    F32 = mybir.dt.float32
    ALU = mybir.AluOpType

    from concourse.dve_spec import AluOp

    vop = _make_scan_bp_op()
    gop = _make_pick_op("VITERBI_GATHER", AluOp.ADD)
    aop = _make_pick_op("VITERBI_ARGMAX", AluOp.MAX)

    n_ex = PB * NBLK
    nc = bacc.Bacc("TRN2", target_bir_lowering=False, debug=False)

    # xin is pre-reversed along the tag axis (kappa coords) on the host.
    U8 = mybir.dt.uint8
    xin_d = nc.dram_tensor("xin", [n_ex, t_total, N], F32, kind="ExternalInput")
    tpad_d = nc.dram_tensor("tpad", [PB, N * NP1], F32, kind="ExternalInput")
    iota1_d = nc.dram_tensor("iota1", [PB, N], F32, kind="ExternalInput")
    iotan_d = nc.dram_tensor("iotan", [PB, N], F32, kind="ExternalInput")
    inval_d = nc.dram_tensor("inval", [n_ex, t_total], U8, kind="ExternalInput")
    e0chunk_d = nc.dram_tensor("e0chunk", [PB, CHO * N], F32, kind="ExternalInput")
    out_d = nc.dram_tensor("out", [n_ex, t_total, N], F32, kind="ExternalOutput")

    bph_d = [nc.dram_tensor(f"bph{g}", [PB, L[g] * N], F32) for g in range(NBLK)]

    with tile.TileContext(nc) as tc:
        with (
            tc.tile_pool(name="const", bufs=1) as cpool,
            tc.tile_pool(name="msk", bufs=NBLK) as mpool,
            tc.tile_pool(name="xs", bufs=6) as xpool,
            tc.tile_pool(name="sp", bufs=6) as spool,
            tc.tile_pool(name="sc", bufs=4) as scpool,
            tc.tile_pool(name="bi", bufs=5) as bipool,
            tc.tile_pool(name="oc", bufs=5) as opool,
            tc.tile_pool(name="ds", bufs=3) as dspool,
            tc.tile_pool(name="sm", bufs=16) as smpool,
            tc.tile_pool(name="fs", bufs=NBLK) as fspool,
        ):
            tpad = cpool.tile([PB, N * NP1], F32, tag="tpad")
            iota1 = cpool.tile([PB, N], F32, tag="iota1")
            iotan = cpool.tile([PB, N], F32, tag="iotan")
            e0c = cpool.tile([PB, CHO * N], F32, tag="e0c")
            nc.sync.dma_start(out=tpad[:], in_=tpad_d.ap())
            nc.sync.dma_start(out=iota1[:], in_=iota1_d.ap())
            nc.sync.dma_start(out=iotan[:], in_=iotan_d.ap())
            nc.sync.dma_start(out=e0c[:], in_=e0chunk_d.ap())
            tpad3 = tpad[:].rearrange("p (s n) -> p s n", n=NP1)

            # windowed u8 freeze masks per block (1 where step >= seq_len)
            masks = []
            for g in range(NBLK):
                W = L[g] - minL[g]
                if W <= 0:
                    masks.append(None)
                    continue
                t_ = mpool.tile([PB, W], U8, tag="miv", name="miv")
                nc.sync.dma_start(
                    out=t_[:],
                    in_=inval_d.ap()[g * PB : (g + 1) * PB, minL[g] : L[g]],
                )
                masks.append(t_)

            def ivcol(g, t):
                return masks[g][:, t - minL[g] : t - minL[g] + 1]

            # padded output region t in [L[g], t_total): one-hot(0) via DMA
            for g in range(NBLK):
                out_g = out_d.ap()[g * PB : (g + 1) * PB]
                t = L[g]
                while t < t_total:
                    t1 = min(t + CHO, t_total)
                    nc.sync.dma_start(
                        out=out_g[:, t:t1].rearrange("p t n -> p (t n)"),
                        in_=e0c[:, : (t1 - t) * N],
                    )
                    t = t1

            # s tiles carry a 49th pad slot; zero it once per buffer. Layout
            # guarantees the pool hands the same buffers back cyclically.
            s_bufs = 6
            pre = [spool.tile([PB, NP1], F32, tag="s", name="s") for _ in range(s_bufs)]
            for t_ in pre:
                nc.gpsimd.memset(t_[:, N:NP1], 0.0)

            fstates = [None] * NBLK

            # ---------------- forward generator ----------------
            def fwd_gen(g):
                Lg, mLg = L[g], minL[g]
                xin_g = xin_d.ap()[g * PB : (g + 1) * PB]
                nchunk = (Lg + CHS - 1) // CHS
                xchunks = [None] * nchunk

                def ensure_chunk(c):
                    if c >= nchunk or xchunks[c] is not None:
                        return
                    t0, t1 = c * CHS, min((c + 1) * CHS, Lg)
                    xt = xpool.tile([PB, CHS * N], F32, tag="x", name="x")
                    nc.sync.dma_start(
                        out=xt[:, : (t1 - t0) * N],
                        in_=xin_g[:, t0:t1].rearrange("p t n -> p (t n)"),
                    )
                    xchunks[c] = xt

                for c in range(min(3, nchunk)):
                    ensure_chunk(c)

                def xcol(t):
                    c, o = divmod(t, CHS)
                    return xchunks[c][:, o * N : (o + 1) * N]

                s_prev = spool.tile([PB, NP1], F32, tag="s", name="s")
                nc.vector.tensor_copy(out=s_prev[:, :N], in_=xcol(0))

                for t in range(1, Lg):
                    sc = scpool.tile([PB, N * NP1], F32, tag="sc", name="sc")
                    # transposed out AP: stream elem (page a, pos c) lands at
                    # offset c*48+a, so the M row (c=47) and bp row (c=48)
                    # are contiguous [PB, 48] slices.
                    scT = sc[:].rearrange("p (c a) -> p a c", c=NP1, a=N)
                    nc.vector._custom_dve(
                        vop,
                        out=scT,
                        in0=tpad3,
                        in1=s_prev[:]
                        .rearrange("p (o n) -> p o n", o=1)
                        .broadcast_to([PB, N, NP1]),
                        s0=float(NP1),
                    )
                    s_t = spool.tile([PB, NP1], F32, tag="s", name="s")
                    # s_t = M + x_t  (M row contiguous at offset 47*48)
                    nc.gpsimd.tensor_tensor(
                        out=s_t[:, :N],
                        in0=sc[:, (N - 1) * N : N * N],
                        in1=xcol(t),
                        op=ALU.add,
                    )
                    if t >= mLg:
                        # frozen examples keep their previous state (exact)
                        nc.vector.copy_predicated(
                            out=s_t[:, :N],
                            mask=ivcol(g, t).to_broadcast([PB, N]),
                            data=s_prev[:, :N],
                        )
                    # bp row (offset 48*48, contiguous) streams to DRAM
                    nc.sync.dma_start(
                        out=bph_d[g].ap()[:, t * N : (t + 1) * N],
                        in_=sc[:, N * N : N * NP1],
                    )
                    ensure_chunk(t // CHS + 2)
                    s_prev = s_t
                    yield

                fst = fspool.tile([PB, N], F32, tag="fst", name="fst")
                nc.scalar.copy(out=fst[:], in_=s_prev[:, :N])
                fstates[g] = fst

            # ---------------- backtrack generator ----------------
            def bt_gen(g):
                Lg, mLg = L[g], minL[g]
                out_g = out_d.ap()[g * PB : (g + 1) * PB]

                bchunks = {}

                def load_bchunk(c):
                    if c < 0 or c in bchunks:
                        return
                    bt = bipool.tile([PB, CHS * N], F32, tag="bi", name="bi")
                    t0, t1 = c * CHS, min((c + 1) * CHS, Lg)
                    nc.sync.dma_start(
                        out=bt[:, : (t1 - t0) * N],
                        in_=bph_d[g].ap()[:, t0 * N : t1 * N],
                    )
                    bchunks[c] = bt

                def bcol(t):
                    c, o = divmod(t, CHS)
                    load_bchunk(c)
                    return bchunks[c][:, o * N : (o + 1) * N]

                ochunks = {}

                def ocol(t):
                    c, o = divmod(t, CHO)
                    if c not in ochunks:
                        ochunks[c] = opool.tile([PB, CHO * N], F32, tag="o", name="o")
                    return ochunks[c][:, o * N : (o + 1) * N]

                def flush_ochunk(c):
                    t0, t1 = c * CHO, min((c + 1) * CHO, Lg)
                    nc.sync.dma_start(
                        out=out_g[:, t0:t1].rearrange("p t n -> p (t n)"),
                        in_=ochunks[c][:, : (t1 - t0) * N],
                    )

                def emit_col(t, dv):
                    """One-hot column t (j coords) on Scalar (Pool lacks
                    comparison opcodes): |dv - (48-j)| via Abs with
                    per-partition bias, then relu(1 - d) — exact 0/1 for
                    integer-valued fp32. Invalid positions get e0."""
                    ad = smpool.tile([PB, N], F32, tag="ad", name="ad")
                    nc.scalar.activation(
                        out=ad[:],
                        in_=iotan[:],
                        func=mybir.ActivationFunctionType.Abs,
                        bias=dv[:, 0:1],
                        scale=1.0,
                    )
                    nc.scalar.activation(
                        out=ocol(t),
                        in_=ad[:],
                        func=mybir.ActivationFunctionType.Relu,
                        bias=1.0,
                        scale=-1.0,
                    )
                    if t >= mLg:
                        nc.vector.copy_predicated(
                            out=ocol(t),
                            mask=ivcol(g, t).to_broadcast([PB, N]),
                            data=e0c[:, :N],
                        )

                if Lg >= 2:
                    load_bchunk((Lg - 1) // CHS)
                    load_bchunk((Lg - 1) // CHS - 1)
                fst = fstates[g]
                mv = smpool.tile([PB, 8], F32, tag="mv", name="mv")
                nc.vector.max(out=mv[:], in_=fst[:])
                dv = smpool.tile([PB, 1], F32, tag="dv", name="dv")
                scr = dspool.tile([PB, N], F32, tag="dsc", name="dsc")
                nc.vector._custom_dve(
                    aop,
                    out=scr[:],
                    in0=fst[:],
                    in1=iota1[:],
                    s0=mv[:, 0:1],
                    accum_out=dv[:],
                )
                emit_col(Lg - 1, dv)
                if Lg == 1:
                    flush_ochunk(0)
                    return

                for t in range(Lg - 1, 0, -1):
                    # gathered backpointer = next dotval (reversed coords)
                    dvn = smpool.tile([PB, 1], F32, tag="dv", name="dv")
                    scr = dspool.tile([PB, N], F32, tag="dsc", name="dsc")
                    nc.vector._custom_dve(
                        gop,
                        out=scr[:],
                        in0=iota1[:],
                        in1=bcol(t),
                        s0=dv[:, 0:1],
                        accum_out=dvn[:],
                    )
                    if t >= mLg:
                        # frozen: keep previous tag
                        nc.vector.copy_predicated(
                            out=dvn[:], mask=ivcol(g, t), data=dv[:]
                        )
                    dv = dvn
                    emit_col(t - 1, dv)
                    load_bchunk((max(t - 1 - CHS // 2, 0)) // CHS)
                    if t % CHO == 0:
                        flush_ochunk(t // CHO)
                    yield
                flush_ochunk(0)

            # ---------------- pipelined emission ----------------
            _DONE = object()
            active = []

            def pump(yields_per_chain):
                for gen in list(active):
                    for _ in range(yields_per_chain):
                        if next(gen, _DONE) is _DONE:
                            active.remove(gen)
                            break

            # forward pairs co-advance so Pool adds hide behind the paired
            # block's scan; backtracks pump behind everything.
            for ga, gb in ((0, 1), (2, 3)):
                fa, fb = fwd_gen(ga), fwd_gen(gb)
                alive_a = alive_b = True
                while alive_a or alive_b:
                    if alive_a:
                        alive_a = next(fa, _DONE) is not _DONE
                        if not alive_a:
                            active.append(bt_gen(ga))
                        pump(1)
                    if alive_b:
                        alive_b = next(fb, _DONE) is not _DONE
                        if not alive_b:
                            active.append(bt_gen(gb))
                        pump(1)
            while active:
                pump(1)

    nc.compile()
    return nc


_NC_CACHE = {}


def _get_nc(L, minL, t_total):
    key = (tuple(L), tuple(minL), t_total)
    if key not in _NC_CACHE:
        _NC_CACHE[key] = _build_nc(list(L), list(minL), t_total)
    return _NC_CACHE[key]


# --------------------------------------------------------------------------
# host wrapper
# --------------------------------------------------------------------------
def kernel(inputs, transitions, seq_lens, _collect_results=None, _trace=False):
    from concourse.bass_utils import run_bass_kernel_spmd

    inputs = np.ascontiguousarray(np.asarray(inputs, dtype=np.float32))
    transitions = np.ascontiguousarray(np.asarray(transitions, dtype=np.float32))
    seq_lens_in = np.asarray(seq_lens)
    b, t_total, n = inputs.shape
    assert n == N and b == BB and t_total == TT, (inputs.shape,)

    lens = np.clip(seq_lens_in.astype(np.int64), 1, t_total)
    order = np.argsort(-lens, kind="stable")

    # slot s (0..31) holds examples order[s*PB:(s+1)*PB]; core c block g = slot g*8+c
    slots = order.reshape(NCORES * NBLK, PB)
    L = []
    minL = []
    for g in range(NBLK):
        block_lens = lens[slots[g * NCORES : (g + 1) * NCORES].ravel()]
        L.append(int(block_lens.max()))
        minL.append(int(block_lens.min()))

    nc = _get_nc(L, minL, t_total)

    # shared constants (kappa coords: kappa = 47 - i, page a = 47 - j)
    tpad = np.full((N, NP1), np.float32(NEG), dtype=np.float32)
    tpad[:, :N] = transitions[::-1, ::-1].T  # tpad[a, c] = T[47-c, 47-a]
    tpad_b = np.ascontiguousarray(
        np.broadcast_to(tpad.reshape(1, N * NP1), (PB, N * NP1))
    )
    iota1 = np.ascontiguousarray(
        np.broadcast_to(np.arange(1, N + 1, dtype=np.float32)[None], (PB, N))
    )
    iotan = np.ascontiguousarray(
        np.broadcast_to(-(N - np.arange(N)).astype(np.float32)[None], (PB, N))
    )
    e0 = np.zeros((PB, CHO, N), dtype=np.float32)
    e0[:, :, 0] = 1.0
    e0chunk = e0.reshape(PB, CHO * N)

    pos = np.arange(t_total, dtype=np.int64)[None, :]
    in_maps = []
    core_example_idx = []
    for c in range(NCORES):
        idx = np.concatenate([slots[g * NCORES + c] for g in range(NBLK)])
        core_example_idx.append(idx)
        xin = np.ascontiguousarray(inputs[idx][:, :, ::-1])  # reversed tag axis
        inval = (pos >= lens[idx][:, None]).astype(np.uint8)
        in_maps.append(
            {
                "xin": xin,
                "tpad": tpad_b,
                "iota1": iota1,
                "iotan": iotan,
                "inval": np.ascontiguousarray(inval),
                "e0chunk": e0chunk,
            }
        )

    run_kwargs = {}
    if _trace:
        run_kwargs = dict(trace=True, trace_cores=[0])
    res = run_bass_kernel_spmd(nc, in_maps, core_ids=list(range(NCORES)), **run_kwargs)
    if _collect_results is not None:
        _collect_results.append(res)

    out = np.empty((b, t_total, N), dtype=np.float32)
    for c in range(NCORES):
        out[core_example_idx[c]] = res.results[c]["out"]
    return out


# revision 22
# speedup vs baseline: 1.4546x; 1.4546x over previous
"""CRF Viterbi decode kernel for Trainium2 (8 NeuronCores, data-parallel).

Problem: inputs [4096, 512, 48] f32, transitions [48, 48] f32, seq_lens [4096] i32.
Output: one-hot of the Viterbi path, [4096, 512, 48] f32 (bit-exact vs the
fp32 jax reference, including first-index argmax tie-breaks).

Design (v2 — fused backpointer scan)
------------------------------------
- Data parallel over batch: 8 cores x 4 blocks of 128 examples (partitions).
  Examples globally sorted by seq_len (desc); block position g on every core
  holds similar lengths and runs a static step count L[g].
- Forward DP on DVE via ONE custom op per step (VITERBI_SCAN_BP): pages of
  49 streaming the candidates in REVERSED tag coords (kappa = 47 - i,
  page a = 47 - j). Per page the op emits the running max stream, the page
  max M at slot 47, and the FIRST-INDEX argmax (1-based kappa coord) at
  slot 48 — exact first-i tie-break falls out of record-detection on the
  reversed stream. IEEE fp32 adds -> bit-exact scores vs the reference.
- s_t = M + x_t on Pool (GPSIMD); per-(example,step) freeze handled by exact
  multiplicative 0/1 masking on Pool. Backpointer rows stream to DRAM.
- Backtrack: ONE tiny DVE op per step (VITERBI_GATHER): accumulate
  select(iota==dotval, bp_row, 0) -> gathered backpointer, which in the
  reversed coordinate system IS the next dotval directly. One-hot output
  columns built on Pool (is_eq against a reversed iota), so the Vector
  engine runs scans + 48-element gathers only.
- Emission: forward blocks co-advance in pairs so each block's Pool add runs
  in the shadow of the other's scan; backtracks pump between scans.
"""

import sys

sys.path.insert(0, "/opt/trn_rl_repo")

import numpy as np

N = 48
NP1 = N + 1  # padded page width
TT = 512
BB = 4096
NCORES = 8
PB = 128  # examples per block (partitions)
NBLK = 4  # blocks per core
CHS = 32  # x / backpointer chunk (steps)
CHO = 32  # output chunk (steps)
NEG = -3.4e38


# --------------------------------------------------------------------------
# custom DVE ops
# --------------------------------------------------------------------------
def _patch_framework():
    import concourse.dve_spec as ds
    from concourse.dve_spec import AluOp

    # segmented-scan page reset: at each sub-dim boundary the accumulator
    # restarts from the boundary element's expr.
    if not getattr(ds, "_ant_seg_reset_patched", False):
        _orig = ds._scan_overrides

        def _patched(scans, node_stage):
            seed, step = _orig(scans, node_stage)
            for sc in scans:
                if getattr(sc, "_ant_seg_reset", False):
                    d = node_stage[sc]
                    step[d] = ds._Stage(AluOp.BYPASS, sc.expr)
            return seed, step

        ds._scan_overrides = _patched
        ds._ant_seg_reset_patched = True

    # hand placement hook (the stock list scheduler mis-orders the
    # counter scan and inserts a cond shim, overflowing the 8 stages).
    if not getattr(ds, "_ant_sched_patched", False):
        _orig_sched = ds._schedule

        def _sched(body, n_stages):
            hs = getattr(ds, "_ant_hand_schedules", {}).get(id(body))
            if hs is not None:
                return dict(hs[0]), list(hs[1]), {}
            return _orig_sched(body, n_stages)

        ds._schedule = _sched
        ds._ant_sched_patched = True
        ds._ant_hand_schedules = {}


def _raw_scan(op, expr, init=None):
    """Scan node bypassing __post_init__ (its nested-scan validator rejects
    cross-scan dataflow that the datapath supports fine)."""
    from concourse.dve_spec import Scan

    sc = Scan.__new__(Scan)
    object.__setattr__(sc, "op", op)
    object.__setattr__(sc, "expr", expr)
    object.__setattr__(sc, "init", init)
    object.__setattr__(sc, "_subdim_step", None)
    object.__setattr__(sc, "_ant_seg_reset", True)
    return sc


def _register_op(name, spec, subdim):
    import concourse.dve_ops as dops
    from concourse.dve_uop import DveOpSpec
    import concourse.dve_spec as ds

    shas = {}
    for ver in ("v3", "v4"):
        uops = ds.lower(spec, ver=ver)
        shas[ver] = DveOpSpec(
            name=name, opcode=1, uops=uops, rd1_en=dops.has_src1(spec)
        ).sha(ver)
    op = dops.DveOp(name, spec, subdim=subdim, uops_sha=shas)
    dops.OPS.append(op)
    dops.CUSTOM_DVE_SPECS[op.name] = op.spec
    dops._SUB_OPCODE_FOR_NAME[op.name] = dops._CUSTOM_DVE_ROW_BASE + len(dops.OPS) - 1
    return op


def _make_scan_bp_op():
    """Segmented (pages of NP1) op on reversed-coord candidate streams:
      v    = Src0 + Src1                   (T_pad + s broadcast)
      m    = seg-scan(MAX, v)              running page max
      idxc = seg-scan(ADD, 1, init=0)      1..49 within page
      flag = v >= m                        record-or-tie (v == m)
      isel = select(flag, idxc, -3.4e38)
      amax = seg-scan(MAX, isel)           last flagged idxc = first-argmax
      out  = select(idxc >= s0, amax, m)   s0 = 49.0: slot 48 carries amax
    Page layout of out: [m stream ... M at slot 47, bp (1-based kappa) at 48].
    """
    import concourse.dve_ops as dops
    import concourse.dve_spec as ds
    from concourse.dve_spec import Spec, Src0, Src1, C0, One, MaxNeg, AluOp, Bin, Tri

    for op in dops.OPS:
        if op.name == "VITERBI_SCAN_BP":
            return op
    _patch_framework()

    v = Bin(AluOp.ADD, Src0, Src1)
    m = _raw_scan(AluOp.MAX, v)
    idxc = _raw_scan(AluOp.ADD, One, init=Bin(AluOp.SUBTRACT, One, One))
    flag = Bin(AluOp.IS_GE, v, m)
    isel = Tri(AluOp.SELECT, flag, idxc, MaxNeg)
    amax = _raw_scan(AluOp.MAX, isel)
    pos = Bin(AluOp.IS_GE, idxc, C0)
    outsel = Tri(AluOp.SELECT, pos, amax, m)

    ds._ant_hand_schedules[id(outsel)] = (
        {v: 0, m: 1, idxc: 2, flag: 3, isel: 4, amax: 5, pos: 6, outsel: 7},
        [Src0, Src1, One, MaxNeg, C0],
    )

    def _ref(in0, in1, s0, s1, imm2):
        v = (np.asarray(in0, np.float32) + np.asarray(in1, np.float32)).astype(
            np.float32
        )
        m = np.maximum.accumulate(v, axis=-1)
        n = v.shape[-1]
        idxc = np.arange(1, n + 1, dtype=np.float32)
        srt = [1] * (v.ndim - 1) + [n]
        isel = np.where(v >= m, idxc.reshape(srt), np.float32(NEG)).astype(np.float32)
        amax = np.maximum.accumulate(isel, axis=-1)
        return np.where(idxc.reshape(srt) >= np.float32(s0), amax, m).astype(
            np.float32
        )

    return _register_op("VITERBI_SCAN_BP", Spec(body=outsel, reference=_ref), True)


def _make_pick_op(name, accum_op):
    """out-stream = select(Src0 == s0, Src1, 0); accum_out = accum over it.
    With accum=ADD and Src0 an iota matched exactly once: a per-partition
    gather of Src1 at index s0. With accum=MAX: first-index argmax helper.
    """
    import concourse.dve_ops as dops
    from concourse.dve_spec import Spec, Src0, Src1, C0, Zero, AluOp, Bin, Tri

    for op in dops.OPS:
        if op.name == name:
            return op
    _patch_framework()

    sel = Tri(AluOp.SELECT, Bin(AluOp.IS_EQ, Src0, C0), Src1, Zero)

    def _ref(in0, in1, s0, s1, imm2):
        in0 = np.asarray(in0, np.float32)
        in1 = np.asarray(in1, np.float32)
        s0a = np.asarray(s0, np.float32)
        while s0a.ndim < in0.ndim:
            s0a = s0a[..., None]
        out = np.where(in0 == s0a, in1, np.float32(0.0)).astype(np.float32)
        if accum_op == AluOp.ADD:
            acc = out.sum(axis=-1, dtype=np.float32)
        else:
            acc = out.max(axis=-1)
        return out, acc.astype(np.float32)

    return _register_op(name, Spec(body=sel, accum=accum_op, reference=_ref), False)


# --------------------------------------------------------------------------
# device program
# --------------------------------------------------------------------------
def _build_nc(L, minL, t_total):
    import concourse.tile as tile
    import concourse.bacc as bacc
    from concourse import mybir

    F32 = mybir.dt.float32
    ALU = mybir.AluOpType

    from concourse.dve_spec import AluOp

    vop = _make_scan_bp_op()
    gop = _make_pick_op("VITERBI_GATHER", AluOp.ADD)
    aop = _make_pick_op("VITERBI_ARGMAX", AluOp.MAX)

    n_ex = PB * NBLK
    nc = bacc.Bacc("TRN2", target_bir_lowering=False, debug=False)

    # xin is pre-reversed along the tag axis (kappa coords) on the host.
    U8 = mybir.dt.uint8
    xin_d = nc.dram_tensor("xin", [n_ex, t_total, N], F32, kind="ExternalInput")
    tpad_d = nc.dram_tensor("tpad", [PB, N * NP1], F32, kind="ExternalInput")
    iota1_d = nc.dram_tensor("iota1", [PB, N], F32, kind="ExternalInput")
    iotan_d = nc.dram_tensor("iotan", [PB, N], F32, kind="ExternalInput")
    inval_d = nc.dram_tensor("inval", [n_ex, t_total], U8, kind="ExternalInput")
    e0chunk_d = nc.dram_tensor("e0chunk", [PB, CHO * N], F32, kind="ExternalInput")
    out_d = nc.dram_tensor("out", [n_ex, t_total, N], F32, kind="ExternalOutput")

    bph_d = [nc.dram_tensor(f"bph{g}", [PB, L[g] * N], F32) for g in range(NBLK)]

    with tile.TileContext(nc) as tc:
        with (
            tc.tile_pool(name="const", bufs=1) as cpool,
            tc.tile_pool(name="msk", bufs=NBLK) as mpool,
            tc.tile_pool(name="xs", bufs=6) as xpool,
            tc.tile_pool(name="sp", bufs=6) as spool,
            tc.tile_pool(name="sc", bufs=4) as scpool,
            tc.tile_pool(name="bo", bufs=4) as bopool,
            tc.tile_pool(name="bi", bufs=5) as bipool,
            tc.tile_pool(name="oc", bufs=5) as opool,
            tc.tile_pool(name="ds", bufs=3) as dspool,
            tc.tile_pool(name="sm", bufs=16) as smpool,
            tc.tile_pool(name="fs", bufs=NBLK) as fspool,
        ):
            tpad = cpool.tile([PB, N * NP1], F32, tag="tpad")
            iota1 = cpool.tile([PB, N], F32, tag="iota1")
            iotan = cpool.tile([PB, N], F32, tag="iotan")
            e0c = cpool.tile([PB, CHO * N], F32, tag="e0c")
            nc.sync.dma_start(out=tpad[:], in_=tpad_d.ap())
            nc.sync.dma_start(out=iota1[:], in_=iota1_d.ap())
            nc.sync.dma_start(out=iotan[:], in_=iotan_d.ap())
            nc.sync.dma_start(out=e0c[:], in_=e0chunk_d.ap())
            tpad3 = tpad[:].rearrange("p (s n) -> p s n", n=NP1)

            # windowed u8 freeze masks per block (1 where step >= seq_len)
            masks = []
            for g in range(NBLK):
                W = L[g] - minL[g]
                if W <= 0:
                    masks.append(None)
                    continue
                t_ = mpool.tile([PB, W], U8, tag="miv", name="miv")
                nc.sync.dma_start(
                    out=t_[:],
                    in_=inval_d.ap()[g * PB : (g + 1) * PB, minL[g] : L[g]],
                )
                masks.append(t_)

            def ivcol(g, t):
                return masks[g][:, t - minL[g] : t - minL[g] + 1]

            # padded output region t in [L[g], t_total): one-hot(0) via DMA
            for g in range(NBLK):
                out_g = out_d.ap()[g * PB : (g + 1) * PB]
                t = L[g]
                while t < t_total:
                    t1 = min(t + CHO, t_total)
                    nc.sync.dma_start(
                        out=out_g[:, t:t1].rearrange("p t n -> p (t n)"),
                        in_=e0c[:, : (t1 - t) * N],
                    )
                    t = t1

            # s tiles carry a 49th pad slot; zero it once per buffer. Layout
            # guarantees the pool hands the same buffers back cyclically.
            s_bufs = 6
            pre = [spool.tile([PB, NP1], F32, tag="s", name="s") for _ in range(s_bufs)]
            for t_ in pre:
                nc.gpsimd.memset(t_[:, N:NP1], 0.0)

            fstates = [None] * NBLK

            # ---------------- forward generator ----------------
            def fwd_gen(g):
                Lg, mLg = L[g], minL[g]
                xin_g = xin_d.ap()[g * PB : (g + 1) * PB]
                nchunk = (Lg + CHS - 1) // CHS
                xchunks = [None] * nchunk

                def ensure_chunk(c):
                    if c >= nchunk or xchunks[c] is not None:
                        return
                    t0, t1 = c * CHS, min((c + 1) * CHS, Lg)
                    xt = xpool.tile([PB, CHS * N], F32, tag="x", name="x")
                    nc.sync.dma_start(
                        out=xt[:, : (t1 - t0) * N],
                        in_=xin_g[:, t0:t1].rearrange("p t n -> p (t n)"),
                    )
                    xchunks[c] = xt

                for c in range(min(3, nchunk)):
                    ensure_chunk(c)

                def xcol(t):
                    c, o = divmod(t, CHS)
                    return xchunks[c][:, o * N : (o + 1) * N]

                bchunk = [None]  # current bp-out chunk tile

                def bpslot(t):
                    o = t % CHS
                    if o == 0 or bchunk[0] is None:
                        bchunk[0] = bopool.tile([PB, CHS * N], F32, tag="bo", name="bo")
                    return bchunk[0][:, o * N : (o + 1) * N]

                def bpflush(t_last):
                    # steps [c*CHS .. t_last] of chunk c = t_last//CHS
                    c = t_last // CHS
                    t0 = max(c * CHS, 1)
                    nc.sync.dma_start(
                        out=bph_d[g].ap()[:, t0 * N : (t_last + 1) * N],
                        in_=bchunk[0][:, (t0 % CHS) * N : ((t_last % CHS) + 1) * N],
                    )

                s_prev = spool.tile([PB, NP1], F32, tag="s", name="s")
                nc.vector.tensor_copy(out=s_prev[:, :N], in_=xcol(0))

                for t in range(1, Lg):
                    sc = scpool.tile([PB, N * NP1], F32, tag="sc", name="sc")
                    sc3 = sc[:].rearrange("p (s n) -> p s n", n=NP1)
                    nc.vector._custom_dve(
                        vop,
                        out=sc3,
                        in0=tpad3,
                        in1=s_prev[:]
                        .rearrange("p (o n) -> p o n", o=1)
                        .broadcast_to([PB, N, NP1]),
                        s0=float(NP1),
                    )
                    s_t = spool.tile([PB, NP1], F32, tag="s", name="s")
                    # s_t = M + x_t  (M at slot 47 of each page)
                    nc.gpsimd.tensor_tensor(
                        out=s_t[:, :N],
                        in0=sc3[:, :, N - 1 : N].rearrange("p s o -> p (s o)"),
                        in1=xcol(t),
                        op=ALU.add,
                    )
                    if t >= mLg:
                        # frozen examples keep their previous state (exact)
                        nc.vector.copy_predicated(
                            out=s_t[:, :N],
                            mask=ivcol(g, t).to_broadcast([PB, N]),
                            data=s_prev[:, :N],
                        )
                    # stage bp row (slot 48 of each page) into the out chunk
                    nc.scalar.copy(
                        out=bpslot(t),
                        in_=sc3[:, :, N : NP1].rearrange("p s o -> p (s o)"),
                    )
                    if t == Lg - 1 or (t + 1) % CHS == 0:
                        bpflush(t)
                    ensure_chunk(t // CHS + 2)
                    s_prev = s_t
                    yield

                fst = fspool.tile([PB, N], F32, tag="fst", name="fst")
                nc.scalar.copy(out=fst[:], in_=s_prev[:, :N])
                fstates[g] = fst

            # ---------------- backtrack generator ----------------
            def bt_gen(g):
                Lg, mLg = L[g], minL[g]
                out_g = out_d.ap()[g * PB : (g + 1) * PB]

                bchunks = {}

                def load_bchunk(c):
                    if c < 0 or c in bchunks:
                        return
                    bt = bipool.tile([PB, CHS * N], F32, tag="bi", name="bi")
                    t0, t1 = c * CHS, min((c + 1) * CHS, Lg)
                    nc.sync.dma_start(
                        out=bt[:, : (t1 - t0) * N],
                        in_=bph_d[g].ap()[:, t0 * N : t1 * N],
                    )
                    bchunks[c] = bt

                def bcol(t):
                    c, o = divmod(t, CHS)
                    load_bchunk(c)
                    return bchunks[c][:, o * N : (o + 1) * N]

                ochunks = {}

                def ocol(t):
                    c, o = divmod(t, CHO)
                    if c not in ochunks:
                        ochunks[c] = opool.tile([PB, CHO * N], F32, tag="o", name="o")
                    return ochunks[c][:, o * N : (o + 1) * N]

                def flush_ochunk(c):
                    t0, t1 = c * CHO, min((c + 1) * CHO, Lg)
                    nc.sync.dma_start(
                        out=out_g[:, t0:t1].rearrange("p t n -> p (t n)"),
                        in_=ochunks[c][:, : (t1 - t0) * N],
                    )

                def emit_col(t, dv):
                    """One-hot column t (j coords) on Scalar (Pool lacks
                    comparison opcodes): |dv - (48-j)| via Abs with
                    per-partition bias, then relu(1 - d) — exact 0/1 for
                    integer-valued fp32. Invalid positions get e0."""
                    ad = smpool.tile([PB, N], F32, tag="ad", name="ad")
                    nc.scalar.activation(
                        out=ad[:],
                        in_=iotan[:],
                        func=mybir.ActivationFunctionType.Abs,
                        bias=dv[:, 0:1],
                        scale=1.0,
                    )
                    nc.scalar.activation(
                        out=ocol(t),
                        in_=ad[:],
                        func=mybir.ActivationFunctionType.Relu,
                        bias=1.0,
                        scale=-1.0,
                    )
                    if t >= mLg:
                        nc.vector.copy_predicated(
                            out=ocol(t),
                            mask=ivcol(g, t).to_broadcast([PB, N]),
                            data=e0c[:, :N],
                        )

                if Lg >= 2:
                    load_bchunk((Lg - 1) // CHS)
                    load_bchunk((Lg - 1) // CHS - 1)
                fst = fstates[g]
                mv = smpool.tile([PB, 8], F32, tag="mv", name="mv")
                nc.vector.max(out=mv[:], in_=fst[:])
                dv = smpool.tile([PB, 1], F32, tag="dv", name="dv")
                scr = dspool.tile([PB, N], F32, tag="dsc", name="dsc")
                nc.vector._custom_dve(
                    aop,
                    out=scr[:],
                    in0=fst[:],
                    in1=iota1[:],
                    s0=mv[:, 0:1],
                    accum_out=dv[:],
                )
                emit_col(Lg - 1, dv)
                if Lg == 1:
                    flush_ochunk(0)
                    return

                for t in range(Lg - 1, 0, -1):
                    # gathered backpointer = next dotval (reversed coords)
                    dvn = smpool.tile([PB, 1], F32, tag="dv", name="dv")
                    scr = dspool.tile([PB, N], F32, tag="dsc", name="dsc")
                    nc.vector._custom_dve(
                        gop,
                        out=scr[:],
                        in0=iota1[:],
                        in1=bcol(t),
                        s0=dv[:, 0:1],
                        accum_out=dvn[:],
                    )
                    if t >= mLg:
                        # frozen: keep previous tag
                        nc.vector.copy_predicated(
                            out=dvn[:], mask=ivcol(g, t), data=dv[:]
                        )
                    dv = dvn
                    emit_col(t - 1, dv)
                    load_bchunk((max(t - 1 - CHS // 2, 0)) // CHS)
                    if t % CHO == 0:
                        flush_ochunk(t // CHO)
                    yield
                flush_ochunk(0)

            # ---------------- pipelined emission ----------------
            _DONE = object()
            active = []

            def pump(yields_per_chain):
                for gen in list(active):
                    for _ in range(yields_per_chain):
                        if next(gen, _DONE) is _DONE:
                            active.remove(gen)
                            break

            # forward pairs co-advance so Pool adds hide behind the paired
            # block's scan; backtracks pump behind everything.
            for ga, gb in ((0, 1), (2, 3)):
                fa, fb = fwd_gen(ga), fwd_gen(gb)
                alive_a = alive_b = True
                while alive_a or alive_b:
                    if alive_a:
                        alive_a = next(fa, _DONE) is not _DONE
                        if not alive_a:
                            active.append(bt_gen(ga))
                        pump(1)
                    if alive_b:
                        alive_b = next(fb, _DONE) is not _DONE
                        if not alive_b:
                            active.append(bt_gen(gb))
                        pump(1)
            while active:
                pump(1)

    nc.compile()
    return nc


_NC_CACHE = {}


def _get_nc(L, minL, t_total):
    key = (tuple(L), tuple(minL), t_total)
    if key not in _NC_CACHE:
        _NC_CACHE[key] = _build_nc(list(L), list(minL), t_total)
    return _NC_CACHE[key]


# --------------------------------------------------------------------------
# host wrapper
# --------------------------------------------------------------------------
def kernel(inputs, transitions, seq_lens, _collect_results=None, _trace=False):
    from concourse.bass_utils import run_bass_kernel_spmd

    inputs = np.ascontiguousarray(np.asarray(inputs, dtype=np.float32))
    transitions = np.ascontiguousarray(np.asarray(transitions, dtype=np.float32))
    seq_lens_in = np.asarray(seq_lens)
    b, t_total, n = inputs.shape
    assert n == N and b == BB and t_total == TT, (inputs.shape,)

    lens = np.clip(seq_lens_in.astype(np.int64), 1, t_total)
    order = np.argsort(-lens, kind="stable")

    # slot s (0..31) holds examples order[s*PB:(s+1)*PB]; core c block g = slot g*8+c
    slots = order.reshape(NCORES * NBLK, PB)
    L = []
    minL = []
    for g in range(NBLK):
        block_lens = lens[slots[g * NCORES : (g + 1) * NCORES].ravel()]
        L.append(int(block_lens.max()))
        minL.append(int(block_lens.min()))

    nc = _get_nc(L, minL, t_total)

    # shared constants (kappa coords: kappa = 47 - i, page a = 47 - j)
    tpad = np.full((N, NP1), np.float32(NEG), dtype=np.float32)
    tpad[:, :N] = transitions[::-1, ::-1].T  # tpad[a, c] = T[47-c, 47-a]
    tpad_b = np.ascontiguousarray(
        np.broadcast_to(tpad.reshape(1, N * NP1), (PB, N * NP1))
    )
    iota1 = np.ascontiguousarray(
        np.broadcast_to(np.arange(1, N + 1, dtype=np.float32)[None], (PB, N))
    )
    iotan = np.ascontiguousarray(
        np.broadcast_to(-(N - np.arange(N)).astype(np.float32)[None], (PB, N))
    )
    e0 = np.zeros((PB, CHO, N), dtype=np.float32)
    e0[:, :, 0] = 1.0
    e0chunk = e0.reshape(PB, CHO * N)

    pos = np.arange(t_total, dtype=np.int64)[None, :]
    in_maps = []
    core_example_idx = []
    for c in range(NCORES):
        idx = np.concatenate([slots[g * NCORES + c] for g in range(NBLK)])
        core_example_idx.append(idx)
        xin = np.ascontiguousarray(inputs[idx][:, :, ::-1])  # reversed tag axis
        inval = (pos >= lens[idx][:, None]).astype(np.uint8)
        in_maps.append(
            {
                "xin": xin,
                "tpad": tpad_b,
                "iota1": iota1,
                "iotan": iotan,
                "inval": np.ascontiguousarray(inval),
                "e0chunk": e0chunk,
            }
        )

    run_kwargs = {}
    if _trace:
        run_kwargs = dict(trace=True, trace_cores=[0])
    res = run_bass_kernel_spmd(nc, in_maps, core_ids=list(range(NCORES)), **run_kwargs)
    if _collect_results is not None:
        _collect_results.append(res)

    out = np.empty((b, t_total, N), dtype=np.float32)
    for c in range(NCORES):
        out[core_example_idx[c]] = res.results[c]["out"]
    return out
